# revision 31
# baseline (speedup 1.0000x reference)
"""Causal multi-head attention block on 8 trn2 NeuronCores.

Problem: B=2, S=2048, D=768, H=12, Dh=64 (fp32), causal softmax attention
with QKV projections and output projection summed over heads.

Sharding: tensor-parallel over heads x data-parallel over batch.
core c in [0,8): b = c//4, heads = {3g, 3g+1, 3g+2} with g = c%4.
Each core computes the partial output sum over its 3 heads for its batch;
the host sums the 4 partials per batch (the TP all-reduce) and stacks.

Per-core device kernel (v2 schedule):
  - all inputs pre-cast to bf16 and pre-transposed on the host; DMA lands
    directly in persistent SBUF tiles (no on-device cast pass).
  - emission order: QK-proj(j0) -> V-proj(all) -> attention(j0..j3); the
    QK projections for j1..3 and the output projections for j-1 are woven
    into the attention i-loop through a deferred-work queue so the PE
    never idles while ScalarE runs the exp stream.
  - scores computed transposed S^T[sk, sq]; heads 0,1 use the two
    partition halves of QT2/KT2 with explicit PE row-tiles (0,0)/(64,0);
    head 2 (solo) packs TWO sk-tiles per pass using duplicated Q3/K3 in
    partitions 64:128.
  - exp on ScalarE only (scale=1/8 folded), 0/1 triangular mask applied
    multiplicatively on the diagonal blocks on GpSimdE.
  - V kept [sk, 3, 65] with a ones column so the z matmul also yields the
    softmax denominator D; z normalized by DMA-broadcast reciprocal of D.
  - output projection contracts the head pair (K=128) plus the solo head
    (K=64) into a shared PSUM bank; one merged [128,768] store per row
    block.

Biases are all zeros per the problem spec (fill=zeros); b_O is applied on
the host if nonzero (exact). b_Q/b_K/b_V are asserted zero.
"""

import os
import sys
import types
import numpy as np

B, S, D, H, DH = 2, 2048, 768, 12, 64
N_CORES = 8
P = 128
NK = D // P      # 6 contraction chunks
NJ = S // 512    # 4 sq tiles of 512
NI = S // P      # 16 sk tiles of 128
SQT = 512

_PROGRAM = None
LAST_RESULTS = None


def _install_ntff_shim():
    """antenv.axon_hooks is missing in this image; shim it so trace=True works."""
    if "antenv.axon_hooks" in sys.modules:
        return
    try:
        from trn_agent_boot.trn_boot import _ntff_profile_via_ctypes
        m = types.ModuleType("antenv.axon_hooks")
        hook = _ntff_profile_via_ctypes("/opt/axon/libaxon_pjrt.so")
        m.get_axon_ntff_profile_hook = lambda: hook
        m.set_axon_ntff_profile_hook = lambda h: None
        sys.modules["antenv.axon_hooks"] = m
    except Exception:
        pass


def _build_program():
    import concourse.bass as bass
    import concourse.mybir as mybir
    from concourse import bacc
    from concourse.tile import TileContext
    from concourse.bass import ts, ds

    fp32 = mybir.dt.float32
    bf16 = mybir.dt.bfloat16
    fp8 = mybir.dt.float8e4
    Exp = mybir.ActivationFunctionType.Exp
    Mult = mybir.AluOpType.mult
    DoubleRow = mybir.MatmulPerfMode.DoubleRow

    nc = bacc.Bacc("TRN2", target_bir_lowering=False, debug=False,
                   num_devices=N_CORES)

    # weights arrive pre-packed on the host in [p, (o e)] partition-major
    # layout so every input DMA is one contiguous descriptor per partition
    xT = nc.dram_tensor("xT", (D, S), bf16, kind="ExternalInput")
    wq2 = nc.dram_tensor("wq2", (P, NK * 128), bf16, kind="ExternalInput")
    wk2 = nc.dram_tensor("wk2", (P, NK * 128), bf16, kind="ExternalInput")
    wqk3 = nc.dram_tensor("wqk3", (P, NK * 128), bf16, kind="ExternalInput")
    wv = nc.dram_tensor("wv", (P, NK * 192), bf16, kind="ExternalInput")
    wo2 = nc.dram_tensor("wo2", (128, D), bf16, kind="ExternalInput")
    wo3 = nc.dram_tensor("wo3", (DH, D), bf16, kind="ExternalInput")
    maskin = nc.dram_tensor("mask", (P, P), bf16, kind="ExternalInput")
    out = nc.dram_tensor("out", (S, D), bf16, kind="ExternalOutput")
    EXP_SCALE = 0.125

    with TileContext(nc) as tc:
        with tc.tile_pool(name="work", bufs=1) as work, \
             tc.tile_pool(name="epool", bufs=6) as epool, \
             tc.tile_pool(name="zsb", bufs=2) as zsb, \
             tc.tile_pool(name="zcol", bufs=2) as zcol, \
             tc.tile_pool(name="dram", bufs=2, space="DRAM") as dram, \
             tc.tile_pool(name="psum", bufs=2, space="PSUM") as psum:

            # ---------------- persistent SBUF tiles ----------------
            QT2 = work.tile([P, S], bf16, name="QT2")   # h0 rows 0:64, h1 64:128
            KT2 = work.tile([P, S], bf16, name="KT2")
            QT3d = work.tile([P, S], bf16, name="QT3d")  # h2 Q in BOTH halves
            KT3d = work.tile([P, S], bf16, name="KT3d")  # h2 K in BOTH halves
            V_all = work.tile([P, NI, 3, 65], bf16, name="V_all")
            xTb = work.tile([P, NK, S], bf16, name="xTb")
            wq2b = work.tile([P, NK, 128], bf16, name="wq2b")
            wk2b = work.tile([P, NK, 128], bf16, name="wk2b")
            wqk3b = work.tile([P, NK, 128], bf16, name="wqk3b")
            wvb = work.tile([P, NK, 192], bf16, name="wvb")
            wo2b = work.tile([P, D], bf16, name="wo2b")
            wo3b = work.tile([64, D], bf16, name="wo3b")
            maskb = work.tile([P, P], bf16, name="maskb")

            # ---------------- input DMAs (no casts needed) ----------------
            # ordered so the first proj matmul (needs wq2b + x chunk 0) can
            # start as early as possible
            nc.vector.memset(V_all[:, :, :, 64], 1.0)
            onesb = work.tile([1, 64], fp32, name="onesb")
            nc.vector.memset(onesb[:], 1.0)
            nc.sync.dma_start(wq2b[:], wq2[:].rearrange("p (o e) -> p o e", o=NK))
            # x split per chunk into the j0 half (cols 0:512, needed first by
            # the j0 QK proj and V t0..3) and the rest
            nc.sync.dma_start(xTb[:, 0, 0:512], xT[ts(0, P), 0:512])
            nc.sync.dma_start(wk2b[:], wk2[:].rearrange("p (o e) -> p o e", o=NK))
            nc.sync.dma_start(wqk3b[:], wqk3[:].rearrange("p (o e) -> p o e", o=NK))
            for k in range(1, NK):
                nc.sync.dma_start(xTb[:, k, 0:512], xT[ts(k, P), 0:512])
            nc.sync.dma_start(wvb[:], wv[:].rearrange("p (o e) -> p o e", o=NK))
            for k in range(NK):
                nc.sync.dma_start(xTb[:, k, 512:1024], xT[ts(k, P), 512:1024])
            nc.sync.dma_start(maskb[:], maskin[:])
            nc.sync.dma_start(wo2b[:], wo2[:])
            nc.sync.dma_start(wo3b[:], wo3[:])
            for k in range(NK):
                nc.sync.dma_start(xTb[:, k, 1024:S], xT[ts(k, P), 1024:S])

            # ---------------- deferred-work queue ----------------
            workq = []

            def pop_work(n=1):
                for _ in range(n):
                    if workq:
                        workq.pop(0)()

            # ---------------- QK projection chunks for slice j ----------
            # proj psum uses the "o" bank rotation so it never perturbs the
            # attention scores pipeline on the "s" rotation.
            def make_qk_chunks(j, copy_op):
                sl = ts(j, SQT)

                def c_q2():
                    q2p = psum.tile([P, SQT], fp32, tag="o", name="q2p",
                                    bufs=1)
                    for k in range(NK):
                        nc.tensor.matmul(q2p[:], wq2b[:, k, :],
                                         xTb[:, k, sl],
                                         start=(k == 0), stop=(k == NK - 1))
                    copy_op(QT2[:, sl], q2p[:])

                def c_k2():
                    k2p = psum.tile([P, SQT], fp32, tag="o", name="k2p",
                                    bufs=1)
                    for k in range(NK):
                        nc.tensor.matmul(k2p[:], wk2b[:, k, :],
                                         xTb[:, k, sl],
                                         start=(k == 0), stop=(k == NK - 1))
                    copy_op(KT2[:, sl], k2p[:])

                def c_qk3():
                    qk3p = psum.tile([P, SQT], fp32, tag="o", name="qk3p",
                                     bufs=1)
                    for k in range(NK):
                        nc.tensor.matmul(qk3p[:], wqk3b[:, k, :],
                                         xTb[:, k, sl],
                                         start=(k == 0), stop=(k == NK - 1))
                    copy_op(QT3d[0:64, sl], qk3p[0:64, :])
                    copy_op(KT3d[64:128, sl], qk3p[64:128, :])
                    # duplicate into the other partition half for 2-i packing
                    nc.sync.dma_start(QT3d[64:128, sl], QT3d[0:64, sl])
                    nc.sync.dma_start(KT3d[0:64, sl], KT3d[64:128, sl])

                return [c_q2, c_k2, c_qk3]

            # ---------------- V projection chunks ----------------
            def make_v_chunk(t, tag, bufs, copy_op):
                def c_v():
                    vp = psum.tile([P, SQT], fp32, tag=tag, name="vp",
                                   bufs=bufs)
                    for k in range(NK):
                        nc.tensor.matmul(vp[:, 0:192], xTb[:, k, ts(t, P)],
                                         wvb[:, k, :],
                                         start=(k == 0), stop=(k == NK - 1))
                    copy_op(V_all[:, t, :, 0:64],
                            vp[:, 0:192].rearrange("p (h e) -> p h e", h=3))
                return c_v

            # ---------------- j0 QK proj (eager) + V t0..3 ----------------
            for c in make_qk_chunks(0, nc.scalar.copy):
                c()

            # queue all remaining proj work; attention j0 starts right away.
            # V t0..3 land on the "z" banks (before the lazy z-accumulator
            # allocation rotates onto them); V t4..15 and the QK projs share
            # the "o" bank so they never touch the "s" scores rotation or
            # the held "z" accumulators.
            for t in range(2):
                workq.append(make_v_chunk(t, "z", 3, nc.vector.tensor_copy))
            workq.extend(make_qk_chunks(1, nc.vector.tensor_copy))
            for t in range(2, 4):
                workq.append(make_v_chunk(t, "z", 3, nc.vector.tensor_copy))
            for t in range(4, 8):
                workq.append(make_v_chunk(t, "o", 1, nc.vector.tensor_copy))
            workq.extend(make_qk_chunks(2, nc.vector.tensor_copy))
            for t in range(8, 12):
                workq.append(make_v_chunk(t, "o", 1, nc.vector.tensor_copy))
            workq.extend(make_qk_chunks(3, nc.vector.tensor_copy))
            for t in range(12, NI):
                workq.append(make_v_chunk(t, "o", 1, nc.vector.tensor_copy))

            # ---------------- attention ----------------
            def make_out_chunks(j, zT2, zT3, tag="o", bufs=1, tail=False):
                chunks = []
                for c in range(4):
                    stage = zsb.tile([P, D], bf16, tag="ost", name="ost")

                    def c1(c=c, zT2=zT2, zT3=zT3, stage=stage):
                        o1 = psum.tile([P, SQT], fp32, tag=tag, name="o1",
                                       bufs=bufs)
                        nc.tensor.matmul(o1[:], zT2[:, ts(c, P)],
                                         wo2b[:, 0:512], start=True,
                                         stop=False, skip_group_check=True)
                        nc.tensor.matmul(o1[:], zT3[:, ts(c, P)],
                                         wo3b[:, 0:512], start=False,
                                         stop=True, skip_group_check=True)
                        nc.vector.tensor_copy(stage[:, 0:512], o1[:])

                    def c2(j=j, c=c, zT2=zT2, zT3=zT3, stage=stage):
                        o2 = psum.tile([P, SQT], fp32, tag=tag, name="o2",
                                       bufs=bufs)
                        nc.tensor.matmul(o2[:, 0:256], zT2[:, ts(c, P)],
                                         wo2b[:, 512:768], start=True,
                                         stop=False, skip_group_check=True)
                        nc.tensor.matmul(o2[:, 0:256], zT3[:, ts(c, P)],
                                         wo3b[:, 512:768], start=False,
                                         stop=True, skip_group_check=True)
                        if tail:
                            # ScalarE is idle in the endgame; let it drain
                            # the second psum bank in parallel with vector
                            nc.scalar.copy(stage[:, 512:768], o2[:, 0:256])
                        else:
                            nc.vector.tensor_copy(stage[:, 512:768],
                                                  o2[:, 0:256])
                        nc.sync.dma_start(out[ds(SQT * j + P * c, P), :],
                                          stage[:])

                    chunks.append(c1)
                    chunks.append(c2)
                return chunks

            # per-j state created at j start; z matmuls of a group are
            # deferred into the NEXT group (crossing j boundaries) so the
            # scores->exp pipeline never waits on them
            jstate = {}

            def z_emit(j, grp):
                st = jstate[j]
                if not st["z_pss"]:
                    st["z_pss"].extend(
                        psum.tile([P, SQT], fp32, tag="z",
                                  name=f"z_ps{h}", bufs=3)
                        for h in range(3))
                z_pss = st["z_pss"]
                n_i = 4 * j + 4
                Ep0, Ep1, Es, i0, i1, c00, c01 = grp
                for h in range(2):
                    nc.tensor.matmul(
                        z_pss[h][0:65, c00:SQT], V_all[:, i0, h, :],
                        Ep0[:, h, c00:SQT], start=(i0 == 0),
                        stop=False, skip_group_check=True)
                    nc.tensor.matmul(
                        z_pss[h][0:65, c01:SQT], V_all[:, i1, h, :],
                        Ep1[:, h, c01:SQT], start=False,
                        stop=(i1 == n_i - 1), skip_group_check=True)
                nc.tensor.matmul(
                    z_pss[2][0:65, c00:SQT], V_all[:, i0, 2, :],
                    Es[:, 0, c00:SQT], start=(i0 == 0),
                    stop=False, skip_group_check=True)
                nc.tensor.matmul(
                    z_pss[2][0:65, c01:SQT], V_all[:, i1, 2, :],
                    Es[:, 1, c01:SQT], start=False,
                    stop=(i1 == n_i - 1), skip_group_check=True)

            def normalize_emit(j):
                st = jstate[j]
                z_pss, zT2, zT3 = st["z_pss"], st["zT2"], st["zT3"]
                last = (j == NJ - 1)
                dbcs = []
                for h in range(3):
                    drow = zsb.tile([1, SQT], fp32, tag=f"drow{h}",
                                    name="drow")
                    nc.vector.tensor_copy(drow[:], z_pss[h][64:65, :])
                    dinv = zsb.tile([1, SQT], fp32, tag=f"dinv{h}",
                                    name="dinv")
                    nc.vector.reciprocal_approx_fast(dinv[:], drow[:])
                    if last:
                        # attention is over: broadcast on the idle PE into
                        # a free "s" bank instead of the slow DMA bounce
                        # (tensor_tensor cannot take two PSUM operands, so
                        # evacuate to SBUF on the otherwise-idle ScalarE)
                        dbp = psum.tile([64, SQT], fp32, tag="s",
                                        name="dbcp")
                        nc.tensor.matmul(dbp[:], onesb[:], dinv[:],
                                         start=True, stop=True)
                        dbc = zsb.tile([64, SQT], fp32, tag=f"dbc{h}",
                                       name="dbc")
                        nc.scalar.copy(dbc[:], dbp[:])
                    else:
                        dscr = dram.tile([1, SQT], fp32, name="dscr")
                        nc.sync.dma_start(dscr[:], dinv[:])
                        dbc = zsb.tile([64, SQT], fp32, tag=f"dbc{h}",
                                       name="dbc")
                        nc.sync.dma_start(dbc[:],
                                          dscr[:].to_broadcast((64, SQT)))
                    dbcs.append(dbc)

                if not last:
                    nc.vector.tensor_tensor(zT2[0:64, :], z_pss[0][0:64, :],
                                            dbcs[0][:], Mult)
                    z1t = zcol.tile([64, SQT], bf16, tag="z1t", name="z1t")
                    nc.vector.tensor_tensor(z1t[:], z_pss[1][0:64, :],
                                            dbcs[1][:], Mult)
                    nc.sync.dma_start(zT2[64:128, :], z1t[:])
                    nc.vector.tensor_tensor(zT3[:], z_pss[2][0:64, :],
                                            dbcs[2][:], Mult)
                    workq.extend(make_out_chunks(j, zT2, zT3,
                                                 tag="o", bufs=1))
                    return

                # final j: normalize column-by-column and emit the output
                # projection for finished columns immediately, so the tail
                # pipelines normalize/matmul/copy/DMA and the PE never goes
                # cold waiting for the full zT tiles
                def norm_cols(c):
                    sl = ts(c, P)
                    nc.vector.tensor_tensor(zT2[0:64, sl],
                                            z_pss[0][0:64, sl],
                                            dbcs[0][:, sl], Mult)
                    z1t = zcol.tile([64, P], bf16, tag="z1tc", name="z1t")
                    nc.vector.tensor_tensor(z1t[:], z_pss[1][0:64, sl],
                                            dbcs[1][:, sl], Mult)
                    nc.sync.dma_start(zT2[64:128, sl], z1t[:])
                    nc.vector.tensor_tensor(zT3[:, sl], z_pss[2][0:64, sl],
                                            dbcs[2][:, sl], Mult)

                chunks = make_out_chunks(j, zT2, zT3, tag="s", bufs=2,
                                         tail=True)
                norm_cols(0)
                norm_cols(1)
                chunks[0]()
                chunks[1]()
                norm_cols(2)
                chunks[2]()
                chunks[3]()
                norm_cols(3)
                for c in chunks[4:]:
                    c()

            pending = None  # closure: z mms (+ normalize) of prev group

            def make_pending(j, grp, last_of_j):
                def run():
                    z_emit(j, grp)
                    if last_of_j:
                        normalize_emit(j)
                return run

            for j in range(NJ):
                jstate[j] = {
                    "zT2": zcol.tile([P, SQT], bf16, tag="zT2", name="zT2"),
                    "zT3": zcol.tile([64, SQT], bf16, tag="zT3", name="zT3"),
                    "z_pss": [],
                }
                zT2 = jstate[j]["zT2"]
                zT3 = jstate[j]["zT3"]
                n_i = 4 * j + 4
                n_g = n_i // 2
                jsl = ts(j, SQT)

                def col0_of(i):
                    return P * (i - 4 * j) if i >= 4 * j else 0

                for g in range(n_g):
                    i0, i1 = 2 * g, 2 * g + 1
                    c00, c01 = col0_of(i0), col0_of(i1)
                    diag = i1 >= 4 * j

                    # -- pair scores for i0 and i1 (row-tiled heads) --
                    sp0 = psum.tile([P, 2, SQT], fp32, tag="s", name="sp0")
                    nc.tensor.matmul(sp0[:, 0, c00:SQT],
                                     KT2[0:64, ts(i0, P)],
                                     QT2[0:64, ds(SQT * j + c00, SQT - c00)],
                                     start=True, stop=True,
                                     tile_position=(0, 0))
                    nc.tensor.matmul(sp0[:, 1, c00:SQT],
                                     KT2[64:128, ts(i0, P)],
                                     QT2[64:128, ds(SQT * j + c00, SQT - c00)],
                                     start=True, stop=True,
                                     tile_position=(64, 0))
                    Ep0 = epool.tile([P, 2, SQT], bf16, name="Ep0")
                    nc.scalar.activation(Ep0[:, :, c00:SQT],
                                         sp0[:, :, c00:SQT], Exp, scale=EXP_SCALE)
                    if i0 >= 4 * j:
                        nc.vector.tensor_tensor(
                            Ep0[:, :, c00:c00 + P], Ep0[:, :, c00:c00 + P],
                            maskb[:, None, :].to_broadcast((P, 2, P)), Mult)
                    pop_work()

                    sp1 = psum.tile([P, 2, SQT], fp32, tag="s", name="sp1")
                    nc.tensor.matmul(sp1[:, 0, c01:SQT],
                                     KT2[0:64, ts(i1, P)],
                                     QT2[0:64, ds(SQT * j + c01, SQT - c01)],
                                     start=True, stop=True,
                                     tile_position=(0, 0))
                    nc.tensor.matmul(sp1[:, 1, c01:SQT],
                                     KT2[64:128, ts(i1, P)],
                                     QT2[64:128, ds(SQT * j + c01, SQT - c01)],
                                     start=True, stop=True,
                                     tile_position=(64, 0))
                    Ep1 = epool.tile([P, 2, SQT], bf16, name="Ep1")
                    nc.scalar.activation(Ep1[:, :, c01:SQT],
                                         sp1[:, :, c01:SQT], Exp, scale=EXP_SCALE)
                    if diag:
                        nc.vector.tensor_tensor(
                            Ep1[:, :, c01:c01 + P], Ep1[:, :, c01:c01 + P],
                            maskb[:, None, :].to_broadcast((P, 2, P)), Mult)
                    pop_work()

                    # -- solo scores: two sk-tiles in one pass --
                    ss = psum.tile([P, 2, SQT], fp32, tag="s", name="ss")
                    nc.tensor.matmul(ss[:, 0, c00:SQT],
                                     KT3d[0:64, ts(i0, P)],
                                     QT3d[0:64, ds(SQT * j + c00, SQT - c00)],
                                     start=True, stop=True,
                                     tile_position=(0, 0))
                    nc.tensor.matmul(ss[:, 1, c01:SQT],
                                     KT3d[64:128, ts(i1, P)],
                                     QT3d[64:128, ds(SQT * j + c01, SQT - c01)],
                                     start=True, stop=True,
                                     tile_position=(64, 0))
                    Es = epool.tile([P, 2, SQT], bf16, name="Es")
                    if not diag:
                        nc.scalar.activation(Es[:, :, :], ss[:, :, :],
                                             Exp, scale=EXP_SCALE)
                    else:
                        nc.scalar.activation(Es[:, 0, c00:SQT],
                                             ss[:, 0, c00:SQT], Exp,
                                             scale=EXP_SCALE)
                        nc.scalar.activation(Es[:, 1, c01:SQT],
                                             ss[:, 1, c01:SQT], Exp,
                                             scale=EXP_SCALE)
                        nc.vector.tensor_tensor(
                            Es[:, 0, c00:c00 + P], Es[:, 0, c00:c00 + P],
                            maskb[:], Mult)
                        nc.vector.tensor_tensor(
                            Es[:, 1, c01:c01 + P], Es[:, 1, c01:c01 + P],
                            maskb[:], Mult)

                    # -- z matmuls (+ normalize) of the previous group --
                    if pending is not None:
                        pending()
                    else:
                        pop_work(2)
                    pending = make_pending(
                        j, (Ep0, Ep1, Es, i0, i1, c00, c01),
                        last_of_j=(g == n_g - 1))
                    pop_work()

            pending()
            while workq:
                pop_work()

    nc.compile()
    return nc


def _get_program():
    global _PROGRAM
    if _PROGRAM is None:
        _PROGRAM = _build_program()
    return _PROGRAM


def kernel(x, W_Q, W_K, W_V, W_O, b_Q, b_K, b_V, b_O):
    global LAST_RESULTS
    _install_ntff_shim()
    from concourse import bass_utils
    import ml_dtypes

    bfloat16 = ml_dtypes.bfloat16

    x = np.asarray(x, dtype=np.float32)
    W_Q = np.asarray(W_Q, dtype=np.float32)
    W_K = np.asarray(W_K, dtype=np.float32)
    W_V = np.asarray(W_V, dtype=np.float32)
    W_O = np.asarray(W_O, dtype=np.float32)
    b_Q = np.asarray(b_Q, dtype=np.float32)
    b_K = np.asarray(b_K, dtype=np.float32)
    b_V = np.asarray(b_V, dtype=np.float32)
    b_O = np.asarray(b_O, dtype=np.float32)
    assert not (np.any(b_Q) or np.any(b_K) or np.any(b_V)), \
        "kernel assumes zero QKV biases (problem spec fill=zeros)"

    nc = _get_program()

    def bf(a):
        return np.ascontiguousarray(a.astype(bfloat16))

    def pack(w):
        # [(o p), e] -> [p, (o e)]: one contiguous DMA descriptor/partition
        e = w.shape[1]
        return bf(w.reshape(NK, P, e).transpose(1, 0, 2).reshape(P, NK * e))

    mask = bf(np.triu(np.ones((P, P), dtype=np.float32)))
    xTs = [bf(x[b].T) for b in range(B)]

    in_maps = []
    for c in range(N_CORES):
        b, g = c // 4, c % 4
        hs = [3 * g, 3 * g + 1, 3 * g + 2]
        in_maps.append({
            "xT": xTs[b],
            "wq2": pack(np.concatenate([W_Q[hs[0]], W_Q[hs[1]]], axis=1)),
            "wk2": pack(np.concatenate([W_K[hs[0]], W_K[hs[1]]], axis=1)),
            "wqk3": pack(np.concatenate([W_Q[hs[2]], W_K[hs[2]]], axis=1)),
            "wv": pack(np.concatenate([W_V[hs[0]], W_V[hs[1]], W_V[hs[2]]],
                                      axis=1)),
            "wo2": bf(np.concatenate([W_O[hs[0]], W_O[hs[1]]], axis=0)),
            "wo3": bf(W_O[hs[2]]),
            "mask": mask,
        })

    res = bass_utils.run_bass_kernel_spmd(
        nc, in_maps, core_ids=list(range(N_CORES)),
        trace=bool(os.environ.get("BASS_TRACE")))
    LAST_RESULTS = res

    parts = [np.asarray(res.results[c]["out"], dtype=np.float32)
             for c in range(N_CORES)]
    full = np.stack([
        parts[0] + parts[1] + parts[2] + parts[3],
        parts[4] + parts[5] + parts[6] + parts[7],
    ], axis=0)
    if np.any(b_O):
        full = full + b_O
    return full.astype(np.float32)
